# revision 16
# baseline (speedup 1.0000x reference)
"""Windowed multi-head attention (Swin-style) for 8 Trainium2 NeuronCores.

Problem: x [1024, 64, 512], mask [256, 64, 64], H=16 heads, D=32.
Data-parallel over windows: core c gets windows [128c, 128c+128) and mask
shard mask[(c%2)*128 : (c%2)*128+128] (window b uses mask[b % 256]).

All GEMMs run in f16 (fp32r measured ~2 cycles/row on HW vs f16's 1).
Softmax scale is folded into Wq on the host; zero biases are skipped
(ones-row bias matmuls only emitted when a bias is nonzero).

Host: xT [B,C,N] f16; expmaskT2 [B/2, 128, 128] f16 = exp(maskT)
pair-stacked block-diagonal (off-diagonal = exp(-1e4) = 0) so cross-window
score garbage is killed exactly; wq (pre-scaled) / wkv / wp f16.

Per-core dataflow (128 windows, quads of 4, window-pairs inside):
  XT [128(c), 4kc, 256(4w,64t)] f16 <-- DMA
  QT = Wq^T X^T -> f16 [128(4h,32d), mc, 256]; KT likewise
  qt2/kt2: head-rows shifted to partition 0 via SBUF DMA
  V2 = X Wv [128(2w,64t), 512] -> f16
  ST[t] = KT_h^T QT_h (h = 4*mc+t): st [128(2w,64m), 4mc, 128(2w,64n)]
  E = exp(st) -> a_all [128, 4t, 4mc, 128] f16 (one SBUF tile)
  A = E * expmaskT2 (single DVE mul, mask bcast over all 16 head slots;
     garbage blocks -> exact 0)
  sig[t] = ones^T A_t -> sg [128, 4mc, 128]; rec2 band t <- 1/sg[t] so
     rec2[32hq:32hq+32, mc, n] = 1/sig[head 4mc+hq, n]
  OT'_h = (V2_h)^T A_h -> otc [128(4hq,32d), mc, 128] (unnormalized)
  otn2 = otc * rec2 (DVE, normalizes after aggregation) -> f16
  Y = OTn^T Wp -> y f16 (host converts back to fp32)
"""
import sys

sys.path.insert(0, "/opt/trn_rl_repo")

import numpy as np

N = 64
C = 512
H = 16
D = 32
KC = 4
SCALE = D ** -0.5
NW = 128        # windows per core
N_CORES = 8
JUNK = -1e4     # off-diagonal mask fill; exp -> exact 0


def build_attention(tc, y, xt, emask2, wq, wkv, wp, nw, biases=None):
    """Emit the kernel into TileContext tc.

    DRAM APs: y [nw,64,512] f16 out; xt [nw,512,64] f16; emask2
    [nw/2,128,128] f16; wq/wkv/wp f16 natural (wq pre-scaled by SCALE).
    biases: None (all-zero fast path) or dict with bq2/bkvk2 [128,KC] f32
    (bq2 pre-scaled), bkvv/bp1 [1,C] f16 bias rows.
    """
    import concourse.bass as bass
    from concourse import mybir
    from contextlib import ExitStack

    FP32 = mybir.dt.float32
    F16 = mybir.dt.float16

    nc = tc.nc
    assert nw % 4 == 0
    nq = nw // 4

    ctx = ExitStack()
    with ctx:
        consts = ctx.enter_context(tc.tile_pool(name="consts", bufs=1))
        sbuf = ctx.enter_context(tc.tile_pool(name="sbuf", bufs=1))
        ring = ctx.enter_context(tc.tile_pool(name="ring", bufs=2, space="PSUM"))

        # ---- constants -------------------------------------------------
        wq_sb = consts.tile([128, KC, C], F16)
        nc.sync.dma_start(wq_sb, wq.rearrange("(kc p) c -> p kc c", p=128))
        wkv_sb = consts.tile([128, KC, 2 * C], F16)
        nc.sync.dma_start(wkv_sb, wkv.rearrange("(kc p) c -> p kc c", p=128))
        wp_sb = consts.tile([128, KC, C], F16)
        nc.sync.dma_start(wp_sb, wp.rearrange("(kc p) c -> p kc c", p=128))

        ones_a = consts.tile([128, 128], F16)
        nc.vector.memset(ones_a, 1.0)
        ones32 = consts.tile([128, 32], F16)
        nc.vector.memset(ones32, 1.0)

        if biases is not None:
            bqs_sb = consts.tile([128, KC], FP32)
            nc.sync.dma_start(bqs_sb, biases["bq2"][:, 0:KC])
            bkvk_sb = consts.tile([128, KC], FP32)
            nc.sync.dma_start(bkvk_sb, biases["bkvk2"][:, 0:KC])
            bkvv_row = consts.tile([1, C], F16)
            nc.sync.dma_start(bkvv_row, biases["bkvv"][0:1, :])
            bp_row = consts.tile([1, C], F16)
            nc.sync.dma_start(bp_row, biases["bp1"][0:1, :])
            ones_r = consts.tile([1, 128], F16)
            nc.scalar.copy(ones_r, ones_a[0:1, 0:128])

        # exp-mask pair tiles [128(2w,64m), mp, 128(2w,64n)] f16
        nwp = nw // 2
        emask_sb = consts.tile([128, nwp, 128], F16)
        nc.sync.dma_start(emask_sb, emask2.rearrange("mp m n -> m mp n"))

        # ---- main loop over quads (4 windows), software-pipelined:
        # quad q's QKT prologue is emitted before quad q-1's attention
        # pairs, so the tensor queue never stalls on the copy+shift chain.
        def prologue(q):
            w0 = 4 * q
            xt_sb = sbuf.tile([128, KC, 4, N], F16, tag="xt", bufs=3,
                              name=f"xt_sb_{q}")
            for kc in range(KC):
                nc.sync.dma_start(
                    xt_sb[:, kc, :, :],
                    xt[w0:w0 + 4, 128 * kc:128 * kc + 128, :]
                    .rearrange("w p t -> p w t"))
            xtf = xt_sb.rearrange("p kc w t -> p kc (w t)")

            # QT|KT packed [128(4h,32d), mc, 512(=qt 256 | kt 256)] f16
            qkt_sb = sbuf.tile([128, KC, 2, 256], F16, tag="qkt", bufs=2,
                               name=f"qkt_sb_{q}")
            for mc in range(KC):
                qt_ps = ring.tile([128, 256], FP32, tag="ps_qt", bufs=2,
                                  name=f"qt_ps_{q}_{mc}")
                kt_ps = ring.tile([128, 256], FP32, tag="ps_kt", bufs=1,
                                  name=f"kt_ps_{q}_{mc}")
                for kc in range(KC):
                    nc.tensor.matmul(qt_ps,
                                     wq_sb[:, kc, 128 * mc:128 * mc + 128],
                                     xtf[:, kc, :],
                                     start=(kc == 0), stop=(kc == KC - 1))
                    nc.tensor.matmul(kt_ps,
                                     wkv_sb[:, kc, 128 * mc:128 * mc + 128],
                                     xtf[:, kc, :],
                                     start=(kc == 0), stop=(kc == KC - 1))
                if biases is not None:
                    nc.scalar.activation(qkt_sb[:, mc, 0, :], qt_ps,
                                         mybir.ActivationFunctionType.Identity,
                                         bias=bqs_sb[:, mc:mc + 1], scale=1.0)
                    nc.scalar.activation(qkt_sb[:, mc, 1, :], kt_ps,
                                         mybir.ActivationFunctionType.Identity,
                                         bias=bkvk_sb[:, mc:mc + 1], scale=1.0)
                else:
                    nc.vector.tensor_copy(qkt_sb[:, mc, 0, :], qt_ps)
                    nc.vector.tensor_copy(qkt_sb[:, mc, 1, :], kt_ps)

            # head-rows to partition 0: qt2/kt2 [32, 4j, KC*256]
            qt2 = sbuf.tile([32, 4, KC, 256], F16, tag="qt2", bufs=2,
                            name=f"qt2_{q}")
            kt2 = sbuf.tile([32, 4, KC, 256], F16, tag="kt2", bufs=2,
                            name=f"kt2_{q}")
            for j in range(4):
                nc.sync.dma_start(qt2[:, j, :, :],
                                  qkt_sb[32 * j:32 * j + 32, :, 0, :])
                nc.sync.dma_start(kt2[:, j, :, :],
                                  qkt_sb[32 * j:32 * j + 32, :, 1, :])
            return xtf, qt2, kt2

        def pairs(q, state):
            w0 = 4 * q
            xtf, qt2, kt2 = state
            for p in range(2):  # window pairs
                mp = 2 * q + p
                psl = slice(128 * p, 128 * p + 128)  # pair token cols in quad

                # V2 [128(2w,64t), 512c] f16
                v2_ps = ring.tile([128, C], FP32, tag="ps_io", bufs=2,
                                  name=f"v2_ps_{q}_{p}")
                for kc in range(KC):
                    nc.tensor.matmul(v2_ps, xtf[:, kc, psl],
                                     wkv_sb[:, kc, C:2 * C],
                                     start=(kc == 0),
                                     stop=(kc == KC - 1) and biases is None)
                if biases is not None:
                    nc.tensor.matmul(v2_ps, ones_r[0:1, 0:128], bkvv_row,
                                     start=False, stop=True)
                v2_sb = sbuf.tile([128, C], F16, tag="v2", bufs=3,
                                  name=f"v2_sb_{q}_{p}")
                nc.vector.tensor_copy(v2_sb, v2_ps)

                # scores tile t holds heads {4*mc + t}; exp into one tile
                a_all = sbuf.tile([128, 4, KC, 128], F16, tag="a", bufs=2,
                                  name=f"a_all_{q}_{p}")
                for t in range(4):
                    st_ps = ring.tile([128, 4, 128], FP32, tag="ps_st", bufs=3,
                                      name=f"st_ps_{q}_{p}_{t}")
                    for mc in range(4):
                        h = 4 * mc + t
                        nc.tensor.matmul(st_ps[:, mc, :],
                                         kt2[:, t, mc, psl],
                                         qt2[:, t, mc, psl],
                                         tile_position=(0, 0))
                    nc.scalar.activation(a_all[:, t, :, :], st_ps,
                                         mybir.ActivationFunctionType.Exp)
                # A = E * expmask, one DVE op (bcast over all 16 head slots)
                em = emask_sb[:, mp, :]
                em_bc = bass.AP(tensor=em.tensor, offset=em.offset,
                                ap=[em.ap[0], [0, 16], em.ap[-1]])
                af = a_all.rearrange("p t mc n -> p (t mc) n")
                nc.gpsimd.tensor_mul(af, af, em_bc)

                # sig banded into one PSUM tile via column positions:
                # sg2[32t:+32, mc, n] = sig[head 4mc+t, n]; one full-tile recip
                sg2_ps = ring.tile([128, KC, 128], FP32, tag="ps_st", bufs=3,
                                   name=f"sg2_ps_{q}_{p}")
                for t in range(4):
                    nc.tensor.matmul(
                        sg2_ps[32 * t:32 * t + 32, :, :]
                        .rearrange("p a b -> p (a b)"),
                        ones32,
                        a_all[:, t].rearrange("p a b -> p (a b)"),
                        tile_position=(0, 32 * t))
                rec2 = sbuf.tile([128, KC, 128], FP32, tag="rec", bufs=2,
                                 name=f"rec2_{q}_{p}")
                nc.vector.reciprocal_approx_fast(rec2, sg2_ps)

                # OT' chunk-packed in PSUM via column positions:
                # otc [128(4hq,32d), mc, 128(2w,64n)] (unnormalized)
                otc_ps = ring.tile([128, KC, 128], FP32, tag="ps_io", bufs=2,
                                   name=f"otc_ps_{q}_{p}")
                for mc in range(KC):
                    for hq in range(4):
                        h = 4 * mc + hq
                        nc.tensor.matmul(
                            otc_ps[32 * hq:32 * hq + 32, mc, :],
                            v2_sb[:, 32 * h:32 * h + 32],
                            a_all[:, hq, mc, :], tile_position=(0, 32 * hq))
                otn2 = sbuf.tile([128, KC, 128], F16, tag="otn2", bufs=2,
                                 name=f"otn2_{q}_{p}")
                nc.vector.tensor_mul(otn2, otc_ps, rec2)

                # proj: Y [128(2w,64t), 512] f16
                y_ps = ring.tile([128, C], FP32, tag="ps_io", bufs=2,
                                  name=f"y_ps_{q}_{p}")
                for kc in range(KC):
                    nc.tensor.matmul(y_ps, otn2[:, kc, :], wp_sb[:, kc, :],
                                     start=(kc == 0),
                                     stop=(kc == KC - 1) and biases is None)
                if biases is not None:
                    nc.tensor.matmul(y_ps, ones_r[0:1, 0:128], bp_row,
                                     start=False, stop=True)
                y_sb = sbuf.tile([128, C], F16, tag="y", bufs=3,
                                 name=f"y_sb_{q}_{p}")
                nc.vector.tensor_copy(y_sb, y_ps)
                nc.sync.dma_start(
                    y[w0 + 2 * p:w0 + 2 * p + 2].flatten_outer_dims(), y_sb)

        state = None
        for q in range(nq):
            s = prologue(q)
            if state is not None:
                pairs(q - 1, state)
            state = s
        pairs(nq - 1, state)


_CACHE = {}


def _build_module(nw=NW, has_bias=False):
    key = (nw, has_bias)
    if key in _CACHE:
        return _CACHE[key]
    import concourse.tile as tile
    from concourse import bacc, mybir

    FP32 = mybir.dt.float32
    F16 = mybir.dt.float16
    nc = bacc.Bacc("TRN2", target_bir_lowering=False, debug=False)
    d = {}
    shapes = {
        "xt": ([nw, C, N], F16), "emask2": ([nw // 2, 128, 128], F16),
        "wq": ([C, C], F16), "wkv": ([C, 2 * C], F16), "wp": ([C, C], F16),
    }
    if has_bias:
        shapes.update({
            "bq2": ([128, KC], FP32), "bkvk2": ([128, KC], FP32),
            "bkvv": ([1, C], F16), "bp1": ([1, C], F16),
        })
    for name, (shape, dt) in shapes.items():
        d[name] = nc.dram_tensor(name, shape, dt, kind="ExternalInput")
    d_y = nc.dram_tensor("y", [nw, N, C], F16, kind="ExternalOutput")

    with tile.TileContext(nc) as tc:
        biases = ({k: d[k][:] for k in ("bq2", "bkvk2", "bkvv", "bp1")}
                  if has_bias else None)
        build_attention(tc, d_y[:], d["xt"][:], d["emask2"][:], d["wq"][:],
                        d["wkv"][:], d["wp"][:], nw, biases=biases)
    nc.compile()
    _CACHE[key] = nc
    return nc


def make_in_maps(inputs, nw=NW, n_cores=N_CORES):
    """Host-side preprocessing + per-core sharding. Returns (in_maps, has_bias)."""
    x = np.asarray(inputs["x"], dtype=np.float32)
    mask = np.asarray(inputs["mask"], dtype=np.float32)
    xt = np.ascontiguousarray(x.transpose(0, 2, 1)).astype(np.float16)  # [B,C,N]
    maskt = mask.transpose(0, 2, 1)                          # [nW, m, n]
    nmask = maskt.shape[0]
    # pair-stacked block-diagonal exp-mask [nW/2, 128, 128] f16
    em = np.full((nmask // 2, 128, 128), JUNK, dtype=np.float32)
    em[:, 0:64, 0:64] = maskt[0::2]
    em[:, 64:128, 64:128] = maskt[1::2]
    em2 = np.exp(em).astype(np.float16)
    bq = np.asarray(inputs["bq"], dtype=np.float32)
    bkv = np.asarray(inputs["bkv"], dtype=np.float32)
    bp = np.asarray(inputs["bp"], dtype=np.float32)
    has_bias = bool(np.any(bq) or np.any(bkv) or np.any(bp))
    wq = (np.asarray(inputs["Wq"], dtype=np.float32) * SCALE).astype(np.float16)
    wkv = np.asarray(inputs["Wkv"], dtype=np.float32).astype(np.float16)
    wp = np.asarray(inputs["Wp"], dtype=np.float32).astype(np.float16)
    base = {"wq": np.ascontiguousarray(wq), "wkv": np.ascontiguousarray(wkv),
            "wp": np.ascontiguousarray(wp)}
    if has_bias:
        base["bq2"] = np.ascontiguousarray((bq * SCALE).reshape(KC, 128).T)
        base["bkvk2"] = np.ascontiguousarray(bkv[:C].reshape(KC, 128).T)
        base["bkvv"] = np.ascontiguousarray(
            bkv[C:].reshape(1, C).astype(np.float16))
        base["bp1"] = np.ascontiguousarray(bp.reshape(1, C).astype(np.float16))
    in_maps = []
    for c in range(n_cores):
        m0 = ((c * nw) % nmask) // 2
        im = {"xt": np.ascontiguousarray(xt[c * nw:(c + 1) * nw]),
              "emask2": np.ascontiguousarray(em2[m0:m0 + nw // 2])}
        im.update(base)
        in_maps.append(im)
    return in_maps, has_bias


def kernel(**inputs):
    from concourse.bass_utils import run_bass_kernel_spmd

    in_maps, has_bias = make_in_maps(inputs)
    nc = _build_module(has_bias=has_bias)
    res = run_bass_kernel_spmd(nc, in_maps, core_ids=list(range(N_CORES)))
    return np.concatenate([r["y"] for r in res.results],
                          axis=0).astype(np.float32)


# revision 19
# speedup vs baseline: 1.3615x; 1.3615x over previous
"""Windowed multi-head attention (Swin-style) for 8 Trainium2 NeuronCores.

Problem: x [1024, 64, 512], mask [256, 64, 64], H=16 heads, D=32.
Data-parallel over windows: core c gets windows [128c, 128c+128) and mask
shard mask[(c%2)*128 : (c%2)*128+128] (window b uses mask[b % 256]).

All GEMMs run in f16 (fp32r measured ~2 cycles/row on HW vs f16's 1).
Softmax scale is folded into Wq on the host; zero biases are skipped
(ones-row bias matmuls only emitted when a bias is nonzero).

Host: xT [B,C,N] f16; expmaskT2 [B/2, 128, 128] f16 = exp(maskT)
pair-stacked block-diagonal (off-diagonal = exp(-1e4) = 0) so cross-window
score garbage is killed exactly; wq (pre-scaled) / wkv / wp f16.

Per-core dataflow (128 windows, quads of 4, window-pairs inside):
  XT [128(c), 4kc, 256(4w,64t)] f16 <-- DMA
  QT = Wq^T X^T -> f16 [128(4h,32d), mc, 256]; KT likewise
  qt2/kt2: head-rows shifted to partition 0 via SBUF DMA
  V2 = X Wv [128(2w,64t), 512] -> f16
  ST[t] = KT_h^T QT_h (h = 4*mc+t): st [128(2w,64m), 4mc, 128(2w,64n)]
  E = exp(st) -> a_all [128, 4t, 4mc, 128] f16 (one SBUF tile)
  A = E * expmaskT2 (single DVE mul, mask bcast over all 16 head slots;
     garbage blocks -> exact 0)
  sig[t] = ones^T A_t -> sg [128, 4mc, 128]; rec2 band t <- 1/sg[t] so
     rec2[32hq:32hq+32, mc, n] = 1/sig[head 4mc+hq, n]
  OT'_h = (V2_h)^T A_h -> otc [128(4hq,32d), mc, 128] (unnormalized)
  otn2 = otc * rec2 (DVE, normalizes after aggregation) -> f16
  Y = OTn^T Wp -> y f16 (host converts back to fp32)
"""
import sys

sys.path.insert(0, "/opt/trn_rl_repo")

import numpy as np

N = 64
C = 512
H = 16
D = 32
KC = 4
SCALE = D ** -0.5
NW = 128        # windows per core
N_CORES = 8
JUNK = -1e4     # off-diagonal mask fill; exp -> exact 0


def build_attention(tc, y, xt, emask2, wq, wkv, wp, nw, biases=None,
                    fp8=None):
    """Emit the kernel into TileContext tc.

    DRAM APs: y [nw,64,512] f16 out; xt [nw,512,64] f16; emask2
    [nw/2,128,128] f16; wq/wkv/wp f16 natural (wq pre-scaled by SCALE).
    biases: None (all-zero fast path) or dict with bq2/bkvk2 [128,KC] f32
    (bq2 pre-scaled), bkvv/bp1 [1,C] f16 bias rows.
    """
    import concourse.bass as bass
    from concourse import mybir
    from contextlib import ExitStack

    FP32 = mybir.dt.float32
    F16 = mybir.dt.float16
    F8 = mybir.dt.float8e4
    DR = mybir.MatmulPerfMode.DoubleRow

    nc = tc.nc
    assert nw % 4 == 0
    nq = nw // 4

    ctx = ExitStack()
    with ctx:
        consts = ctx.enter_context(tc.tile_pool(name="consts", bufs=1))
        sbuf = ctx.enter_context(tc.tile_pool(name="sbuf", bufs=1))
        ring = ctx.enter_context(tc.tile_pool(name="ring", bufs=2, space="PSUM"))

        # ---- constants -------------------------------------------------
        if fp8 is None:
            wq_sb = consts.tile([128, KC, C], F16)
            nc.sync.dma_start(wq_sb, wq.rearrange("(kc p) c -> p kc c", p=128))
        else:
            wq8_sb = consts.tile([128, KC, C], F8)
            nc.sync.dma_start(wq8_sb,
                              fp8["wq8"].rearrange("(kc p) c -> p kc c", p=128))
            wk8_sb = consts.tile([128, KC, C], F8)
            nc.sync.dma_start(wk8_sb,
                              fp8["wk8"].rearrange("(kc p) c -> p kc c", p=128))
        wkv_sb = consts.tile([128, KC, 2 * C], F16)
        nc.sync.dma_start(wkv_sb, wkv.rearrange("(kc p) c -> p kc c", p=128))
        wp_sb = consts.tile([128, KC, C], F16)
        nc.sync.dma_start(wp_sb, wp.rearrange("(kc p) c -> p kc c", p=128))

        ones_a = consts.tile([128, 128], F16)
        nc.vector.memset(ones_a, 1.0)
        ones32 = consts.tile([128, 32], F16)
        nc.vector.memset(ones32, 1.0)

        if biases is not None:
            bqs_sb = consts.tile([128, KC], FP32)
            nc.sync.dma_start(bqs_sb, biases["bq2"][:, 0:KC])
            bkvk_sb = consts.tile([128, KC], FP32)
            nc.sync.dma_start(bkvk_sb, biases["bkvk2"][:, 0:KC])
            bkvv_row = consts.tile([1, C], F16)
            nc.sync.dma_start(bkvv_row, biases["bkvv"][0:1, :])
            bp_row = consts.tile([1, C], F16)
            nc.sync.dma_start(bp_row, biases["bp1"][0:1, :])
            ones_r = consts.tile([1, 128], F16)
            nc.scalar.copy(ones_r, ones_a[0:1, 0:128])

        # exp-mask pair tiles [128(2w,64m), mp, 128(2w,64n)] f16
        nwp = nw // 2
        emask_sb = consts.tile([128, nwp, 128], F16)
        nc.sync.dma_start(emask_sb, emask2.rearrange("mp m n -> m mp n"))

        # ---- main loop over quads (4 windows), software-pipelined:
        # quad q's QKT prologue is emitted before quad q-1's attention
        # pairs, so the tensor queue never stalls on the copy+shift chain.
        def prologue(q):
            w0 = 4 * q
            xt_sb = sbuf.tile([128, KC, 4, N], F16, tag="xt", bufs=3,
                              name=f"xt_sb_{q}")
            for kc in range(KC):
                nc.sync.dma_start(
                    xt_sb[:, kc, :, :],
                    xt[w0:w0 + 4, 128 * kc:128 * kc + 128, :]
                    .rearrange("w p t -> p w t"))
            xtf = xt_sb.rearrange("p kc w t -> p kc (w t)")
            if fp8 is not None:
                x8_sb = sbuf.tile([128, KC, 4, N], F8, tag="x8", bufs=3,
                                  name=f"x8_sb_{q}")
                for kc in range(KC):
                    nc.sync.dma_start(
                        x8_sb[:, kc, :, :],
                        fp8["x8"][w0:w0 + 4, 128 * kc:128 * kc + 128, :]
                        .rearrange("w p t -> p w t"))
                x8f = x8_sb.rearrange("p kc w t -> p kc (w t)")

            # QT|KT packed [128(4h,32d), mc, 512(=qt 256 | kt 256)] f16
            qkt_sb = sbuf.tile([128, KC, 2, 256], F16, tag="qkt", bufs=2,
                               name=f"qkt_sb_{q}")
            for mc in range(KC):
                qt_ps = ring.tile([128, 256], FP32, tag="ps_qt", bufs=2,
                                  name=f"qt_ps_{q}_{mc}")
                kt_ps = ring.tile([128, 256], FP32, tag="ps_kt", bufs=1,
                                  name=f"kt_ps_{q}_{mc}")
                if fp8 is not None:
                    for kc in (0, 2):
                        nc.tensor.matmul(qt_ps,
                                         wq8_sb[:, kc:kc + 2,
                                                128 * mc:128 * mc + 128],
                                         x8f[:, kc:kc + 2, :],
                                         start=(kc == 0), stop=(kc == 2),
                                         perf_mode=DR)
                        nc.tensor.matmul(kt_ps,
                                         wk8_sb[:, kc:kc + 2,
                                                128 * mc:128 * mc + 128],
                                         x8f[:, kc:kc + 2, :],
                                         start=(kc == 0), stop=(kc == 2),
                                         perf_mode=DR)
                    nc.vector.tensor_scalar_mul(qkt_sb[:, mc, 0, :], qt_ps,
                                                SCALE / 8.0)
                    nc.vector.tensor_scalar_mul(qkt_sb[:, mc, 1, :], kt_ps,
                                                1.0 / 8.0)
                elif biases is not None:
                    for kc in range(KC):
                        nc.tensor.matmul(qt_ps,
                                         wq_sb[:, kc, 128 * mc:128 * mc + 128],
                                         xtf[:, kc, :],
                                         start=(kc == 0), stop=(kc == KC - 1))
                        nc.tensor.matmul(kt_ps,
                                         wkv_sb[:, kc, 128 * mc:128 * mc + 128],
                                         xtf[:, kc, :],
                                         start=(kc == 0), stop=(kc == KC - 1))
                    nc.scalar.activation(qkt_sb[:, mc, 0, :], qt_ps,
                                         mybir.ActivationFunctionType.Identity,
                                         bias=bqs_sb[:, mc:mc + 1], scale=1.0)
                    nc.scalar.activation(qkt_sb[:, mc, 1, :], kt_ps,
                                         mybir.ActivationFunctionType.Identity,
                                         bias=bkvk_sb[:, mc:mc + 1], scale=1.0)
                else:
                    for kc in range(KC):
                        nc.tensor.matmul(qt_ps,
                                         wq_sb[:, kc, 128 * mc:128 * mc + 128],
                                         xtf[:, kc, :],
                                         start=(kc == 0), stop=(kc == KC - 1))
                        nc.tensor.matmul(kt_ps,
                                         wkv_sb[:, kc, 128 * mc:128 * mc + 128],
                                         xtf[:, kc, :],
                                         start=(kc == 0), stop=(kc == KC - 1))
                    nc.vector.tensor_copy(qkt_sb[:, mc, 0, :], qt_ps)
                    nc.vector.tensor_copy(qkt_sb[:, mc, 1, :], kt_ps)

            # head-rows to partition 0: qt2/kt2 [32, 4j, KC*256]
            qt2 = sbuf.tile([32, 4, KC, 256], F16, tag="qt2", bufs=2,
                            name=f"qt2_{q}")
            kt2 = sbuf.tile([32, 4, KC, 256], F16, tag="kt2", bufs=2,
                            name=f"kt2_{q}")
            for j in range(4):
                nc.sync.dma_start(qt2[:, j, :, :],
                                  qkt_sb[32 * j:32 * j + 32, :, 0, :])
                nc.sync.dma_start(kt2[:, j, :, :],
                                  qkt_sb[32 * j:32 * j + 32, :, 1, :])
            return xtf, qt2, kt2

        def pairs(q, state):
            w0 = 4 * q
            xtf, qt2, kt2 = state
            for p in range(2):  # window pairs
                mp = 2 * q + p
                psl = slice(128 * p, 128 * p + 128)  # pair token cols in quad

                # V2 [128(2w,64t), 512c] f16
                v2_ps = ring.tile([128, C], FP32, tag="ps_io", bufs=2,
                                  name=f"v2_ps_{q}_{p}")
                for kc in range(KC):
                    nc.tensor.matmul(v2_ps, xtf[:, kc, psl],
                                     wkv_sb[:, kc, C:2 * C],
                                     start=(kc == 0),
                                     stop=(kc == KC - 1) and biases is None)
                if biases is not None:
                    nc.tensor.matmul(v2_ps, ones_r[0:1, 0:128], bkvv_row,
                                     start=False, stop=True)
                v2_sb = sbuf.tile([128, C], F16, tag="v2", bufs=3,
                                  name=f"v2_sb_{q}_{p}")
                nc.vector.tensor_copy(v2_sb, v2_ps)

                # scores tile t holds heads {4*mc + t}; exp into one tile
                a_all = sbuf.tile([128, 4, KC, 128], F16, tag="a", bufs=2,
                                  name=f"a_all_{q}_{p}")
                for t in range(4):
                    st_ps = ring.tile([128, 4, 128], FP32, tag="ps_st", bufs=3,
                                      name=f"st_ps_{q}_{p}_{t}")
                    for mc in range(4):
                        h = 4 * mc + t
                        nc.tensor.matmul(st_ps[:, mc, :],
                                         kt2[:, t, mc, psl],
                                         qt2[:, t, mc, psl],
                                         tile_position=(0, 0))
                    nc.scalar.activation(a_all[:, t, :, :], st_ps,
                                         mybir.ActivationFunctionType.Exp)
                # A = E * expmask, one DVE op (bcast over all 16 head slots)
                em = emask_sb[:, mp, :]
                em_bc = bass.AP(tensor=em.tensor, offset=em.offset,
                                ap=[em.ap[0], [0, 16], em.ap[-1]])
                af = a_all.rearrange("p t mc n -> p (t mc) n")
                nc.vector.tensor_mul(af, af, em_bc)

                # sig banded into one PSUM tile via column positions:
                # sg2[32t:+32, mc, n] = sig[head 4mc+t, n]; one full-tile recip
                sg2_ps = ring.tile([128, KC, 128], FP32, tag="ps_st", bufs=3,
                                   name=f"sg2_ps_{q}_{p}")
                for t in range(4):
                    nc.tensor.matmul(
                        sg2_ps[32 * t:32 * t + 32, :, :]
                        .rearrange("p a b -> p (a b)"),
                        ones32,
                        a_all[:, t].rearrange("p a b -> p (a b)"),
                        tile_position=(0, 32 * t))
                rec2 = sbuf.tile([128, KC, 128], FP32, tag="rec", bufs=2,
                                 name=f"rec2_{q}_{p}")
                nc.vector.reciprocal_approx_fast(rec2, sg2_ps)

                # OT' chunk-packed in PSUM via column positions:
                # otc [128(4hq,32d), mc, 128(2w,64n)] (unnormalized)
                otc_ps = ring.tile([128, KC, 128], FP32, tag="ps_io", bufs=2,
                                   name=f"otc_ps_{q}_{p}")
                for mc in range(KC):
                    for hq in range(4):
                        h = 4 * mc + hq
                        nc.tensor.matmul(
                            otc_ps[32 * hq:32 * hq + 32, mc, :],
                            v2_sb[:, 32 * h:32 * h + 32],
                            a_all[:, hq, mc, :], tile_position=(0, 32 * hq))
                otn2 = sbuf.tile([128, KC, 128], F16, tag="otn2", bufs=2,
                                 name=f"otn2_{q}_{p}")
                nc.vector.tensor_mul(otn2, otc_ps, rec2)

                # proj: Y [128(2w,64t), 512] f16
                y_ps = ring.tile([128, C], FP32, tag="ps_io", bufs=2,
                                  name=f"y_ps_{q}_{p}")
                for kc in range(KC):
                    nc.tensor.matmul(y_ps, otn2[:, kc, :], wp_sb[:, kc, :],
                                     start=(kc == 0),
                                     stop=(kc == KC - 1) and biases is None)
                if biases is not None:
                    nc.tensor.matmul(y_ps, ones_r[0:1, 0:128], bp_row,
                                     start=False, stop=True)
                y_sb = sbuf.tile([128, C], F16, tag="y", bufs=3,
                                 name=f"y_sb_{q}_{p}")
                nc.vector.tensor_copy(y_sb, y_ps)
                nc.sync.dma_start(
                    y[w0 + 2 * p:w0 + 2 * p + 2].flatten_outer_dims(), y_sb)

        state = None
        for q in range(nq):
            s = prologue(q)
            if state is not None:
                pairs(q - 1, state)
            state = s
        pairs(nq - 1, state)


_CACHE = {}


def _build_module(nw=NW, has_bias=False, fp8_qk=False):
    key = (nw, has_bias, fp8_qk)
    if key in _CACHE:
        return _CACHE[key]
    import concourse.tile as tile
    from concourse import bacc, mybir

    FP32 = mybir.dt.float32
    F16 = mybir.dt.float16
    F8 = mybir.dt.float8e4
    nc = bacc.Bacc("TRN2", target_bir_lowering=False, debug=False)
    d = {}
    shapes = {
        "xt": ([nw, C, N], F16), "emask2": ([nw // 2, 128, 128], F16),
        "wq": ([C, C], F16), "wkv": ([C, 2 * C], F16), "wp": ([C, C], F16),
    }
    if has_bias:
        shapes.update({
            "bq2": ([128, KC], FP32), "bkvk2": ([128, KC], FP32),
            "bkvv": ([1, C], F16), "bp1": ([1, C], F16),
        })
    if fp8_qk:
        del shapes["wq"]
        shapes.update({
            "x8": ([nw, C, N], F8), "wq8": ([C, C], F8), "wk8": ([C, C], F8),
        })
    for name, (shape, dt) in shapes.items():
        d[name] = nc.dram_tensor(name, shape, dt, kind="ExternalInput")
    d_y = nc.dram_tensor("y", [nw, N, C], F16, kind="ExternalOutput")

    with tile.TileContext(nc) as tc:
        biases = ({k: d[k][:] for k in ("bq2", "bkvk2", "bkvv", "bp1")}
                  if has_bias else None)
        fp8 = ({k: d[k][:] for k in ("x8", "wq8", "wk8")} if fp8_qk else None)
        build_attention(tc, d_y[:], d["xt"][:], d["emask2"][:],
                        d["wq"][:] if not fp8_qk else None,
                        d["wkv"][:], d["wp"][:], nw, biases=biases, fp8=fp8)
    nc.compile()
    _CACHE[key] = nc
    return nc


FP8_QK = False


def make_in_maps(inputs, nw=NW, n_cores=N_CORES):
    """Host-side preprocessing + per-core sharding. Returns (in_maps, has_bias)."""
    x = np.asarray(inputs["x"], dtype=np.float32)
    mask = np.asarray(inputs["mask"], dtype=np.float32)
    xt = np.ascontiguousarray(x.transpose(0, 2, 1)).astype(np.float16)  # [B,C,N]
    maskt = mask.transpose(0, 2, 1)                          # [nW, m, n]
    nmask = maskt.shape[0]
    # pair-stacked block-diagonal exp-mask [nW/2, 128, 128] f16
    em = np.full((nmask // 2, 128, 128), JUNK, dtype=np.float32)
    em[:, 0:64, 0:64] = maskt[0::2]
    em[:, 64:128, 64:128] = maskt[1::2]
    em2 = np.exp(em).astype(np.float16)
    bq = np.asarray(inputs["bq"], dtype=np.float32)
    bkv = np.asarray(inputs["bkv"], dtype=np.float32)
    bp = np.asarray(inputs["bp"], dtype=np.float32)
    has_bias = bool(np.any(bq) or np.any(bkv) or np.any(bp))
    fp8_qk = FP8_QK and not has_bias
    wq = (np.asarray(inputs["Wq"], dtype=np.float32) * SCALE).astype(np.float16)
    wkv = np.asarray(inputs["Wkv"], dtype=np.float32).astype(np.float16)
    wp = np.asarray(inputs["Wp"], dtype=np.float32).astype(np.float16)
    base = {"wq": np.ascontiguousarray(wq), "wkv": np.ascontiguousarray(wkv),
            "wp": np.ascontiguousarray(wp)}
    if fp8_qk:
        import ml_dtypes
        F8NP = ml_dtypes.float8_e4m3fn
        del base["wq"]
        base["wq8"] = np.ascontiguousarray(
            (np.asarray(inputs["Wq"], np.float32) * 8.0).astype(F8NP))
        base["wk8"] = np.ascontiguousarray(
            (np.asarray(inputs["Wkv"], np.float32)[:, :C] * 8.0).astype(F8NP))
    if has_bias:
        base["bq2"] = np.ascontiguousarray((bq * SCALE).reshape(KC, 128).T)
        base["bkvk2"] = np.ascontiguousarray(bkv[:C].reshape(KC, 128).T)
        base["bkvv"] = np.ascontiguousarray(
            bkv[C:].reshape(1, C).astype(np.float16))
        base["bp1"] = np.ascontiguousarray(bp.reshape(1, C).astype(np.float16))
    if fp8_qk:
        import ml_dtypes
        x8 = xt.astype(ml_dtypes.float8_e4m3fn)
    in_maps = []
    for c in range(n_cores):
        m0 = ((c * nw) % nmask) // 2
        im = {"xt": np.ascontiguousarray(xt[c * nw:(c + 1) * nw]),
              "emask2": np.ascontiguousarray(em2[m0:m0 + nw // 2])}
        if fp8_qk:
            im["x8"] = np.ascontiguousarray(x8[c * nw:(c + 1) * nw])
        im.update(base)
        in_maps.append(im)
    return in_maps, (has_bias, fp8_qk)


def kernel(**inputs):
    from concourse.bass_utils import run_bass_kernel_spmd

    in_maps, (has_bias, fp8_qk) = make_in_maps(inputs)
    nc = _build_module(has_bias=has_bias, fp8_qk=fp8_qk)
    res = run_bass_kernel_spmd(nc, in_maps, core_ids=list(range(N_CORES)))
    return np.concatenate([r["y"] for r in res.results],
                          axis=0).astype(np.float32)


# revision 21
# speedup vs baseline: 1.3881x; 1.0195x over previous
"""Windowed multi-head attention (Swin-style) for 8 Trainium2 NeuronCores.

Problem: x [1024, 64, 512], mask [256, 64, 64], H=16 heads, D=32.
Data-parallel over windows: core c gets windows [128c, 128c+128) and mask
shard mask[(c%2)*128 : (c%2)*128+128] (window b uses mask[b % 256]).

All GEMMs run in f16 (fp32r measured ~2 cycles/row on HW vs f16's 1).
Softmax scale is folded into Wq on the host; zero biases are skipped
(ones-row bias matmuls only emitted when a bias is nonzero).

Host: xT [B,C,N] f16; expmaskT2 [B/2, 128, 128] f16 = exp(maskT)
pair-stacked block-diagonal (off-diagonal = exp(-1e4) = 0) so cross-window
score garbage is killed exactly; wq (pre-scaled) / wkv / wp f16.

Per-core dataflow (128 windows, quads of 4, window-pairs inside):
  XT [128(c), 4kc, 256(4w,64t)] f16 <-- DMA
  QT = Wq^T X^T -> f16 [128(4h,32d), mc, 256]; KT likewise
  qt2/kt2: head-rows shifted to partition 0 via SBUF DMA
  V2 = X Wv [128(2w,64t), 512] -> f16
  ST[t] = KT_h^T QT_h (h = 4*mc+t): st [128(2w,64m), 4mc, 128(2w,64n)]
  E = exp(st) -> a_all [128, 4t, 4mc, 128] f16 (one SBUF tile)
  A = E * expmaskT2 (single DVE mul, mask bcast over all 16 head slots;
     garbage blocks -> exact 0)
  sig[t] = ones^T A_t -> sg [128, 4mc, 128]; rec2 band t <- 1/sg[t] so
     rec2[32hq:32hq+32, mc, n] = 1/sig[head 4mc+hq, n]
  OT'_h = (V2_h)^T A_h -> otc [128(4hq,32d), mc, 128] (unnormalized)
  otn2 = otc * rec2 (DVE, normalizes after aggregation) -> f16
  Y = OTn^T Wp -> y f16 (host converts back to fp32)
"""
import sys

sys.path.insert(0, "/opt/trn_rl_repo")

import numpy as np

N = 64
C = 512
H = 16
D = 32
KC = 4
SCALE = D ** -0.5
NW = 128        # windows per core
N_CORES = 8
JUNK = -1e4     # off-diagonal mask fill; exp -> exact 0


def build_attention(tc, y, xt, emask2, wq, wkv, wp, nw, biases=None,
                    fp8=None):
    """Emit the kernel into TileContext tc.

    DRAM APs: y [nw,64,512] f16 out; xt [nw,512,64] f16; emask2
    [nw/2,128,128] f16; wq/wkv/wp f16 natural (wq pre-scaled by SCALE).
    biases: None (all-zero fast path) or dict with bq2/bkvk2 [128,KC] f32
    (bq2 pre-scaled), bkvv/bp1 [1,C] f16 bias rows.
    """
    import concourse.bass as bass
    from concourse import mybir
    from contextlib import ExitStack

    FP32 = mybir.dt.float32
    F16 = mybir.dt.float16
    F8 = mybir.dt.float8e4
    DR = mybir.MatmulPerfMode.DoubleRow

    nc = tc.nc
    assert nw % 4 == 0
    nq = nw // 4

    ctx = ExitStack()
    with ctx:
        consts = ctx.enter_context(tc.tile_pool(name="consts", bufs=1))
        sbuf = ctx.enter_context(tc.tile_pool(name="sbuf", bufs=1))
        ring = ctx.enter_context(tc.tile_pool(name="ring", bufs=2, space="PSUM"))

        # ---- constants -------------------------------------------------
        if fp8 is None:
            wq_sb = consts.tile([128, KC, C], F16)
            nc.sync.dma_start(wq_sb, wq.rearrange("(kc p) c -> p kc c", p=128))
        else:
            wq8_sb = consts.tile([128, KC, C], F8)
            nc.sync.dma_start(wq8_sb,
                              fp8["wq8"].rearrange("(kc p) c -> p kc c", p=128))
            wk8_sb = consts.tile([128, KC, C], F8)
            nc.sync.dma_start(wk8_sb,
                              fp8["wk8"].rearrange("(kc p) c -> p kc c", p=128))
        wkv_sb = consts.tile([128, KC, 2 * C], F16)
        nc.sync.dma_start(wkv_sb, wkv.rearrange("(kc p) c -> p kc c", p=128))
        wp_sb = consts.tile([128, KC, C], F16)
        nc.sync.dma_start(wp_sb, wp.rearrange("(kc p) c -> p kc c", p=128))

        ones_a = consts.tile([128, 128], F16)
        nc.vector.memset(ones_a, 1.0)
        ones32 = consts.tile([128, 32], F16)
        nc.vector.memset(ones32, 1.0)

        if biases is not None:
            bqs_sb = consts.tile([128, KC], FP32)
            nc.sync.dma_start(bqs_sb, biases["bq2"][:, 0:KC])
            bkvk_sb = consts.tile([128, KC], FP32)
            nc.sync.dma_start(bkvk_sb, biases["bkvk2"][:, 0:KC])
            bkvv_row = consts.tile([1, C], F16)
            nc.sync.dma_start(bkvv_row, biases["bkvv"][0:1, :])
            bp_row = consts.tile([1, C], F16)
            nc.sync.dma_start(bp_row, biases["bp1"][0:1, :])
            ones_r = consts.tile([1, 128], F16)
            nc.scalar.copy(ones_r, ones_a[0:1, 0:128])

        # exp-mask pair tiles [128(2w,64m), mp, 128(2w,64n)] f16
        nwp = nw // 2
        emask_sb = consts.tile([128, nwp, 128], F16)
        nc.sync.dma_start(emask_sb, emask2.rearrange("mp m n -> m mp n"))

        # ---- main loop over quads (4 windows), software-pipelined:
        # quad q's QKT prologue is emitted before quad q-1's attention
        # pairs, so the tensor queue never stalls on the copy+shift chain.
        def prologue(q):
            w0 = 4 * q
            xt_sb = sbuf.tile([128, KC, 4, N], F16, tag="xt", bufs=3,
                              name=f"xt_sb_{q}")
            for kc in range(KC):
                nc.sync.dma_start(
                    xt_sb[:, kc, :, :],
                    xt[w0:w0 + 4, 128 * kc:128 * kc + 128, :]
                    .rearrange("w p t -> p w t"))
            xtf = xt_sb.rearrange("p kc w t -> p kc (w t)")
            if fp8 is not None:
                x8_sb = sbuf.tile([128, KC, 4, N], F8, tag="x8", bufs=3,
                                  name=f"x8_sb_{q}")
                for kc in range(KC):
                    nc.sync.dma_start(
                        x8_sb[:, kc, :, :],
                        fp8["x8"][w0:w0 + 4, 128 * kc:128 * kc + 128, :]
                        .rearrange("w p t -> p w t"))
                x8f = x8_sb.rearrange("p kc w t -> p kc (w t)")

            # QT|KT packed [128(4h,32d), mc, 512(=qt 256 | kt 256)] f16
            qkt_sb = sbuf.tile([128, KC, 2, 256], F16, tag="qkt", bufs=2,
                               name=f"qkt_sb_{q}")
            for mc in range(KC):
                qt_ps = ring.tile([128, 256], FP32, tag="ps_qt", bufs=1,
                                  name=f"qt_ps_{q}_{mc}")
                kt_ps = ring.tile([128, 256], FP32, tag="ps_kt", bufs=1,
                                  name=f"kt_ps_{q}_{mc}")
                if fp8 is not None:
                    for kc in (0, 2):
                        nc.tensor.matmul(qt_ps,
                                         wq8_sb[:, kc:kc + 2,
                                                128 * mc:128 * mc + 128],
                                         x8f[:, kc:kc + 2, :],
                                         start=(kc == 0), stop=(kc == 2),
                                         perf_mode=DR)
                        nc.tensor.matmul(kt_ps,
                                         wk8_sb[:, kc:kc + 2,
                                                128 * mc:128 * mc + 128],
                                         x8f[:, kc:kc + 2, :],
                                         start=(kc == 0), stop=(kc == 2),
                                         perf_mode=DR)
                    nc.vector.tensor_scalar_mul(qkt_sb[:, mc, 0, :], qt_ps,
                                                SCALE / 8.0)
                    nc.vector.tensor_scalar_mul(qkt_sb[:, mc, 1, :], kt_ps,
                                                1.0 / 8.0)
                elif biases is not None:
                    for kc in range(KC):
                        nc.tensor.matmul(qt_ps,
                                         wq_sb[:, kc, 128 * mc:128 * mc + 128],
                                         xtf[:, kc, :],
                                         start=(kc == 0), stop=(kc == KC - 1))
                        nc.tensor.matmul(kt_ps,
                                         wkv_sb[:, kc, 128 * mc:128 * mc + 128],
                                         xtf[:, kc, :],
                                         start=(kc == 0), stop=(kc == KC - 1))
                    nc.scalar.activation(qkt_sb[:, mc, 0, :], qt_ps,
                                         mybir.ActivationFunctionType.Identity,
                                         bias=bqs_sb[:, mc:mc + 1], scale=1.0)
                    nc.scalar.activation(qkt_sb[:, mc, 1, :], kt_ps,
                                         mybir.ActivationFunctionType.Identity,
                                         bias=bkvk_sb[:, mc:mc + 1], scale=1.0)
                else:
                    for kc in range(KC):
                        nc.tensor.matmul(qt_ps,
                                         wq_sb[:, kc, 128 * mc:128 * mc + 128],
                                         xtf[:, kc, :],
                                         start=(kc == 0), stop=(kc == KC - 1))
                        nc.tensor.matmul(kt_ps,
                                         wkv_sb[:, kc, 128 * mc:128 * mc + 128],
                                         xtf[:, kc, :],
                                         start=(kc == 0), stop=(kc == KC - 1))
                    nc.scalar.copy(qkt_sb[:, mc, 0, :], qt_ps)
                    nc.vector.tensor_copy(qkt_sb[:, mc, 1, :], kt_ps)

            # head-rows to partition 0: qt2/kt2 [32, 4j, KC*256]
            qt2 = sbuf.tile([32, 4, KC, 256], F16, tag="qt2", bufs=2,
                            name=f"qt2_{q}")
            kt2 = sbuf.tile([32, 4, KC, 256], F16, tag="kt2", bufs=2,
                            name=f"kt2_{q}")
            for j in range(4):
                nc.sync.dma_start(qt2[:, j, :, :],
                                  qkt_sb[32 * j:32 * j + 32, :, 0, :])
                nc.sync.dma_start(kt2[:, j, :, :],
                                  qkt_sb[32 * j:32 * j + 32, :, 1, :])
            return xtf, qt2, kt2

        def pairs(q, state):
            w0 = 4 * q
            xtf, qt2, kt2 = state
            for p in range(2):  # window pairs
                mp = 2 * q + p
                psl = slice(128 * p, 128 * p + 128)  # pair token cols in quad

                # V2 [128(2w,64t), 512c] f16
                v2_ps = ring.tile([128, C], FP32, tag="ps_io", bufs=3,
                                  name=f"v2_ps_{q}_{p}")
                for kc in range(KC):
                    nc.tensor.matmul(v2_ps, xtf[:, kc, psl],
                                     wkv_sb[:, kc, C:2 * C],
                                     start=(kc == 0),
                                     stop=(kc == KC - 1) and biases is None)
                if biases is not None:
                    nc.tensor.matmul(v2_ps, ones_r[0:1, 0:128], bkvv_row,
                                     start=False, stop=True)
                v2_sb = sbuf.tile([128, C], F16, tag="v2", bufs=3,
                                  name=f"v2_sb_{q}_{p}")
                nc.scalar.copy(v2_sb, v2_ps)

                # scores tile t holds heads {4*mc + t}; exp into one tile
                a_all = sbuf.tile([128, 4, KC, 128], F16, tag="a", bufs=2,
                                  name=f"a_all_{q}_{p}")
                for t in range(4):
                    st_ps = ring.tile([128, 4, 128], FP32, tag="ps_st",
                                      name=f"st_ps_{q}_{p}_{t}")
                    for mc in range(4):
                        h = 4 * mc + t
                        nc.tensor.matmul(st_ps[:, mc, :],
                                         kt2[:, t, mc, psl],
                                         qt2[:, t, mc, psl],
                                         tile_position=(0, 0))
                    nc.scalar.activation(a_all[:, t, :, :], st_ps,
                                         mybir.ActivationFunctionType.Exp)
                # A = E * expmask, one DVE op (bcast over all 16 head slots)
                em = emask_sb[:, mp, :]
                em_bc = bass.AP(tensor=em.tensor, offset=em.offset,
                                ap=[em.ap[0], [0, 16], em.ap[-1]])
                af = a_all.rearrange("p t mc n -> p (t mc) n")
                nc.vector.tensor_mul(af, af, em_bc)

                # sig banded into one PSUM tile via column positions:
                # sg2[32t:+32, mc, n] = sig[head 4mc+t, n]; one full-tile recip
                sg2_ps = ring.tile([128, KC, 128], FP32, tag="ps_sg", bufs=1,
                                   name=f"sg2_ps_{q}_{p}")
                for t in range(4):
                    nc.tensor.matmul(
                        sg2_ps[32 * t:32 * t + 32, :, :]
                        .rearrange("p a b -> p (a b)"),
                        ones32,
                        a_all[:, t].rearrange("p a b -> p (a b)"),
                        tile_position=(0, 32 * t))
                rec2 = sbuf.tile([128, KC, 128], FP32, tag="rec", bufs=2,
                                 name=f"rec2_{q}_{p}")
                nc.vector.reciprocal_approx_fast(rec2, sg2_ps)

                # OT' chunk-packed in PSUM via column positions:
                # otc [128(4hq,32d), mc, 128(2w,64n)] (unnormalized)
                otc_ps = ring.tile([128, KC, 128], FP32, tag="ps_io", bufs=3,
                                   name=f"otc_ps_{q}_{p}")
                for mc in range(KC):
                    for hq in range(4):
                        h = 4 * mc + hq
                        nc.tensor.matmul(
                            otc_ps[32 * hq:32 * hq + 32, mc, :],
                            v2_sb[:, 32 * h:32 * h + 32],
                            a_all[:, hq, mc, :], tile_position=(0, 32 * hq))
                otn2 = sbuf.tile([128, KC, 128], F16, tag="otn2", bufs=2,
                                 name=f"otn2_{q}_{p}")
                nc.vector.tensor_mul(otn2, otc_ps, rec2)

                # proj: Y [128(2w,64t), 512] f16
                y_ps = ring.tile([128, C], FP32, tag="ps_io", bufs=3,
                                  name=f"y_ps_{q}_{p}")
                for kc in range(KC):
                    nc.tensor.matmul(y_ps, otn2[:, kc, :], wp_sb[:, kc, :],
                                     start=(kc == 0),
                                     stop=(kc == KC - 1) and biases is None)
                if biases is not None:
                    nc.tensor.matmul(y_ps, ones_r[0:1, 0:128], bp_row,
                                     start=False, stop=True)
                y_sb = sbuf.tile([128, C], F16, tag="y", bufs=3,
                                 name=f"y_sb_{q}_{p}")
                nc.vector.tensor_copy(y_sb, y_ps)
                nc.sync.dma_start(
                    y[w0 + 2 * p:w0 + 2 * p + 2].flatten_outer_dims(), y_sb)

        for q in range(nq):
            pairs(q, prologue(q))


_CACHE = {}


def _build_module(nw=NW, has_bias=False, fp8_qk=False):
    key = (nw, has_bias, fp8_qk)
    if key in _CACHE:
        return _CACHE[key]
    import concourse.tile as tile
    from concourse import bacc, mybir

    FP32 = mybir.dt.float32
    F16 = mybir.dt.float16
    F8 = mybir.dt.float8e4
    nc = bacc.Bacc("TRN2", target_bir_lowering=False, debug=False)
    d = {}
    shapes = {
        "xt": ([nw, C, N], F16), "emask2": ([nw // 2, 128, 128], F16),
        "wq": ([C, C], F16), "wkv": ([C, 2 * C], F16), "wp": ([C, C], F16),
    }
    if has_bias:
        shapes.update({
            "bq2": ([128, KC], FP32), "bkvk2": ([128, KC], FP32),
            "bkvv": ([1, C], F16), "bp1": ([1, C], F16),
        })
    if fp8_qk:
        del shapes["wq"]
        shapes.update({
            "x8": ([nw, C, N], F8), "wq8": ([C, C], F8), "wk8": ([C, C], F8),
        })
    for name, (shape, dt) in shapes.items():
        d[name] = nc.dram_tensor(name, shape, dt, kind="ExternalInput")
    d_y = nc.dram_tensor("y", [nw, N, C], F16, kind="ExternalOutput")

    with tile.TileContext(nc) as tc:
        biases = ({k: d[k][:] for k in ("bq2", "bkvk2", "bkvv", "bp1")}
                  if has_bias else None)
        fp8 = ({k: d[k][:] for k in ("x8", "wq8", "wk8")} if fp8_qk else None)
        build_attention(tc, d_y[:], d["xt"][:], d["emask2"][:],
                        d["wq"][:] if not fp8_qk else None,
                        d["wkv"][:], d["wp"][:], nw, biases=biases, fp8=fp8)
    nc.compile()
    _CACHE[key] = nc
    return nc


FP8_QK = True


def make_in_maps(inputs, nw=NW, n_cores=N_CORES):
    """Host-side preprocessing + per-core sharding. Returns (in_maps, has_bias)."""
    x = np.asarray(inputs["x"], dtype=np.float32)
    mask = np.asarray(inputs["mask"], dtype=np.float32)
    xt = np.ascontiguousarray(x.transpose(0, 2, 1)).astype(np.float16)  # [B,C,N]
    maskt = mask.transpose(0, 2, 1)                          # [nW, m, n]
    nmask = maskt.shape[0]
    # pair-stacked block-diagonal exp-mask [nW/2, 128, 128] f16
    em = np.full((nmask // 2, 128, 128), JUNK, dtype=np.float32)
    em[:, 0:64, 0:64] = maskt[0::2]
    em[:, 64:128, 64:128] = maskt[1::2]
    em2 = np.exp(em).astype(np.float16)
    bq = np.asarray(inputs["bq"], dtype=np.float32)
    bkv = np.asarray(inputs["bkv"], dtype=np.float32)
    bp = np.asarray(inputs["bp"], dtype=np.float32)
    has_bias = bool(np.any(bq) or np.any(bkv) or np.any(bp))
    fp8_qk = FP8_QK and not has_bias
    wq = (np.asarray(inputs["Wq"], dtype=np.float32) * SCALE).astype(np.float16)
    wkv = np.asarray(inputs["Wkv"], dtype=np.float32).astype(np.float16)
    wp = np.asarray(inputs["Wp"], dtype=np.float32).astype(np.float16)
    base = {"wq": np.ascontiguousarray(wq), "wkv": np.ascontiguousarray(wkv),
            "wp": np.ascontiguousarray(wp)}
    if fp8_qk:
        import ml_dtypes
        F8NP = ml_dtypes.float8_e4m3fn
        del base["wq"]
        base["wq8"] = np.ascontiguousarray(
            (np.asarray(inputs["Wq"], np.float32) * 8.0).astype(F8NP))
        base["wk8"] = np.ascontiguousarray(
            (np.asarray(inputs["Wkv"], np.float32)[:, :C] * 8.0).astype(F8NP))
    if has_bias:
        base["bq2"] = np.ascontiguousarray((bq * SCALE).reshape(KC, 128).T)
        base["bkvk2"] = np.ascontiguousarray(bkv[:C].reshape(KC, 128).T)
        base["bkvv"] = np.ascontiguousarray(
            bkv[C:].reshape(1, C).astype(np.float16))
        base["bp1"] = np.ascontiguousarray(bp.reshape(1, C).astype(np.float16))
    if fp8_qk:
        import ml_dtypes
        x8 = xt.astype(ml_dtypes.float8_e4m3fn)
    in_maps = []
    for c in range(n_cores):
        m0 = ((c * nw) % nmask) // 2
        im = {"xt": np.ascontiguousarray(xt[c * nw:(c + 1) * nw]),
              "emask2": np.ascontiguousarray(em2[m0:m0 + nw // 2])}
        if fp8_qk:
            im["x8"] = np.ascontiguousarray(x8[c * nw:(c + 1) * nw])
        im.update(base)
        in_maps.append(im)
    return in_maps, (has_bias, fp8_qk)


def kernel(**inputs):
    from concourse.bass_utils import run_bass_kernel_spmd

    in_maps, (has_bias, fp8_qk) = make_in_maps(inputs)
    nc = _build_module(has_bias=has_bias, fp8_qk=fp8_qk)
    res = run_bass_kernel_spmd(nc, in_maps, core_ids=list(range(N_CORES)))
    return np.concatenate([r["y"] for r in res.results],
                          axis=0).astype(np.float32)
